# revision 1
# baseline (speedup 1.0000x reference)
"""Trainium2 Bass kernel for nn_Correlation (FlowNet-style cost volume).

Problem: input1/input2 [8, 256, 96, 128] f32 ->
         out [8, 441, 96, 128] f32
  out[b, 21*i+j, h, w] = leaky_relu_0.1( (1/256) * sum_c
        in1[b,c,h,w] * in2pad[b,c, h+2i, w+2j] )       (pad 20 each side)

Strategy (data-parallel over B across 8 cores; per core = 1 sample):
  * Displacements are even (dilation 2): pixel (h,w) only correlates with
    in2 pixels of the same (h%2, w%2) parity class. In parity space the
    dilated 21x21 patch is a dense 21x21 window.
  * Per parity class: split the 48x64 parity image into 8x16 pixel
    blocks (128 pixels = PE stationary operand). Stream the block's
    28x36 in2 parity window through the PE contracting over C=256
    (bf16, fp32 PSUM) -> band[pixel, window_col] (1008 cols, 441 useful).
  * PE operands need single-stride APs, so both inputs are rearranged
    on-chip (GPSIMD copies): in1 into parity-blocked contiguous 128-pixel
    groups; in2 into per-wb "bands" where each block's window rows are
    contiguous (36-row rolling buffer over padded parity rows).
  * Band -> SBUF (DVE) -> DRAM scratch (contiguous) -> diagonal gather
    back (per-pixel 21x21 patch; DRAM-side APs are flat so the diagonal
    is legal) -> ALIGNED[pixel, 441].
  * PE-transpose ALIGNED -> [d, pixel]; ScalarE applies
    leaky_relu(x/256) scattering into parity-interleaved row tiles;
    contiguous stores.
"""

import numpy as np

import concourse.bass as bass
import concourse.mybir as mybir
from concourse.tile import TileContext
from concourse.bass_utils import run_bass_kernel_spmd
from concourse.masks import make_identity

DT = mybir.dt

# ---- problem geometry ----
B, C, H, W = 8, 256, 96, 128
NP = 21                      # displacements per axis
ND = NP * NP                 # 441
CC = 2                       # C chunks of 128
HE, WE = H // 2, W // 2      # parity image 48 x 64
HEP, WEP = HE + 20, WE + 20  # padded parity image 68 x 84

HB, WB = 8, 16               # parity block (he, we); HB*WB = 128
WIN_H, WIN_W = HB + 20, WB + 20   # 28 x 36 window
FB = WIN_H * WIN_W           # 1008 band columns
HW = H * W                   # 12288

# in2 "bands": per (cc, hp, wp, wb) a [NSLOT, 36] contiguous-row image,
# rolling over padded parity rows (h'e in [0, 68), slot = h'e % NSLOT).
# NSLOT=40 (not 36) so a prefetched group only overwrites rows of
# ALREADY-FINISHED block-rows (dependency slack for pipelining).
NSLOT = 40
BAND_PITCH = NSLOT * WIN_W               # 1296
N_BANDS = CC * 2 * 2 * 4                 # 32
BANDS_F = N_BANDS * BAND_PITCH           # 41472

IN1BLK_F = 2 * 2 * CC * 4 * 128          # 4096 per block-row slab
STG_F = CC * 16 * W                      # 4096 (16 full-res rows)

_MAX_WAITS = 1


def _split_excess_waits(nc):
    """This walrus build accepts only ONE sync-wait per instruction; Tile
    emits multi-waits. Hoist excess waits onto same-engine NOPs inserted
    right before the over-subscribed instruction."""
    nid = 0
    for f in nc.m.functions:
        for blk in f.blocks:
            insts = list(blk.instructions)
            out = []
            changed = False
            for inst in insts:
                si = inst.sync_info
                if si is not None and si.on_wait and len(si.on_wait) > _MAX_WAITS:
                    waits = list(si.on_wait)
                    extra, keep = waits[:-_MAX_WAITS], waits[-_MAX_WAITS:]
                    for k in range(0, len(extra), _MAX_WAITS):
                        nop = mybir.InstNoOp(name=f"I-waitsplit-{nid}", ins=[], outs=[])
                        nid += 1
                        nop.engine = inst.engine
                        nop.sync_info = mybir.SyncInfo(
                            on_wait=extra[k : k + _MAX_WAITS], on_update=[]
                        )
                        out.append(nop)
                        changed = True
                    si.on_wait = keep
                    inst.sync_info = si
                out.append(inst)
            if changed:
                blk.instructions = out
    return nc


def _ap(t, off_extra, dims):
    return bass.AP(tensor=t.tensor, offset=t.offset + off_extra, ap=dims)


def _band_base(cc, hp, wp, wb):
    return (((cc * 2 + hp) * 2 + wp) * 4 + wb) * BAND_PITCH


def _slot_runs(lo, hi):
    """Contiguous (slot, h'e, count) runs for padded parity rows [lo, hi)."""
    runs = []
    r = lo
    while r < hi:
        s = r % NSLOT
        n = min(hi - r, NSLOT - s)
        runs.append((s, r, n))
        r += n
    return runs


def _row_pieces(a):
    """Matmul N-pieces for block-row a: [(i0, ni), ...] respecting the
    rolling-slot wrap and the 504-column PSUM bank split."""
    s0 = a % NSLOT
    w = NSLOT - s0
    runs = [(0, 28)] if w >= 28 else [(0, w), (w, 28 - w)]
    pieces = []
    for i0, n in runs:
        end = i0 + n
        for b0, b1 in ((0, 14), (14, 28)):
            lo, hi = max(i0, b0), min(end, b1)
            if lo < hi:
                pieces.append((lo, hi - lo))
    return pieces


def _build_nc(debug=False, waitsplit=True):
    nc = bass.Bass()
    in1_d = nc.dram_tensor("in1", [C, H, W], DT.float32, kind="ExternalInput")
    in2_d = nc.dram_tensor("in2", [C, H, W], DT.float32, kind="ExternalInput")
    out_d = nc.dram_tensor("out", [ND, H, W], DT.float32, kind="ExternalOutput")
    dbg = {}
    if debug:
        dbg["bands"] = nc.dram_tensor(
            "dbg_bands", [128, BANDS_F], DT.bfloat16, kind="ExternalOutput"
        )
        dbg["in1blk"] = nc.dram_tensor(
            "dbg_in1blk", [128, IN1BLK_F], DT.bfloat16, kind="ExternalOutput"
        )
        dbg["band_sb"] = nc.dram_tensor(
            "dbg_band_sb", [128, FB], DT.float32, kind="ExternalOutput"
        )
        dbg["alig"] = nc.dram_tensor(
            "dbg_alig", [128, ND], DT.float32, kind="ExternalOutput"
        )
        dbg["tr"] = nc.dram_tensor(
            "dbg_tr", [128, 512], DT.float32, kind="ExternalOutput"
        )
        dbg["bands2"] = nc.dram_tensor(
            "dbg_bands2", [128, BANDS_F], DT.bfloat16, kind="ExternalOutput"
        )
        dbg["band_sb2"] = nc.dram_tensor(
            "dbg_band_sb2", [128, FB], DT.float32, kind="ExternalOutput"
        )
        dbg["alig2"] = nc.dram_tensor(
            "dbg_alig2", [128, ND], DT.float32, kind="ExternalOutput"
        )

    with TileContext(nc) as tc:
        with (
            tc.tile_pool(name="constp", bufs=1) as constp,
            tc.tile_pool(name="bandsp", bufs=1) as bandsp,
            tc.tile_pool(name="stgp", bufs=2) as stgp,
            tc.tile_pool(name="in1p", bufs=2) as in1p,
            tc.tile_pool(name="bandsbp", bufs=2) as bandsbp,
            tc.tile_pool(name="aligp", bufs=3) as aligp,
            tc.tile_pool(name="outp", bufs=1) as outp,
            tc.tile_pool(name="relup", bufs=2) as relup,
            tc.tile_pool(name="psp", bufs=2, space="PSUM") as psp,
            tc.tile_pool(name="trpp", bufs=2, space="PSUM") as trpp,
            tc.tile_pool(name="dramp", bufs=4, space="DRAM") as dramp,
        ):
            identity = constp.tile([128, 128], DT.float32)
            make_identity(nc, identity)

            bands = constp.tile([128, BANDS_F], DT.bfloat16)

            # zero everything once: covers w-padding columns and all
            # initial padding rows (contiguous write = precise dep tracking)
            nc.vector.memset(bands[:, :], 0.0)

            def build_group(g):
                """Fill band rows for padded parity rows [8g, 8g+8)."""
                glo, ghi = 8 * g, min(8 * g + 8, HEP)
                # zero spans (padding rows) - skip for g<2 (initial memset
                # covered them); needed when slots are being recycled
                for zlo, zhi in ((glo, min(ghi, 10)), (max(glo, 58), ghi)):
                    if zlo >= zhi or zhi <= NSLOT:
                        continue
                    for s0, _, n in _slot_runs(zlo, zhi):
                        for cc in range(CC):
                            for hp in range(2):
                                for wp in range(2):
                                    for wb in range(4):
                                        nc.vector.memset(
                                            _ap(
                                                bands,
                                                _band_base(cc, hp, wp, wb)
                                                + s0 * WIN_W,
                                                [[BANDS_F, 128], [1, n * WIN_W]],
                                            ),
                                            0.0,
                                        )
                # data span
                dlo, dhi = max(glo, 10), min(ghi, 58)
                if dlo >= dhi:
                    return
                h0, nh = 2 * (dlo - 10), 2 * (dhi - dlo)
                stg = stgp.tile([128, STG_F], DT.bfloat16, name="stg", bufs=1)
                for cc in range(CC):
                    nc.gpsimd.dma_start(
                        _ap(stg, cc * 16 * W, [[STG_F, 128], [1, nh * W]]),
                        in2_d[cc * 128 : (cc + 1) * 128, h0 : h0 + nh, :],
                    )
                for cc in range(CC):
                    for hp in range(2):
                        for wp in range(2):
                            for s0, he0, n in _slot_runs(dlo, dhi):
                                src_r = 2 * (he0 - dlo) + hp
                                for wb in range(4):
                                    # valid u range for this wb (w-padding)
                                    u0 = 10 if wb == 0 else 0
                                    u1 = 26 if wb == 3 else WIN_W
                                    nu = u1 - u0
                                    s_ap = _ap(
                                        stg,
                                        cc * 16 * W
                                        + src_r * W
                                        + (2 * (16 * wb + u0 - 10) + wp),
                                        [[STG_F, 128], [2 * W, n], [2, nu]],
                                    )
                                    d_ap = _ap(
                                        bands,
                                        _band_base(cc, hp, wp, wb)
                                        + s0 * WIN_W
                                        + u0,
                                        [[BANDS_F, 128], [WIN_W, n], [1, nu]],
                                    )
                                    nc.gpsimd.tensor_copy(d_ap, s_ap)

            def build_in1_slab(ai):
                """Parity-blocked in1 for block-row ai -> [128, IN1BLK_F]."""
                stg1 = stgp.tile([128, STG_F], DT.bfloat16, name="stg1", bufs=1)
                for cc in range(CC):
                    nc.gpsimd.dma_start(
                        _ap(stg1, cc * 16 * W, [[STG_F, 128], [1, 16 * W]]),
                        in1_d[cc * 128 : (cc + 1) * 128, 16 * ai : 16 * ai + 16, :],
                    )
                blk = in1p.tile([128, IN1BLK_F], DT.bfloat16, name="in1blk")
                for cc in range(CC):
                    for hp in range(2):
                        for wp in range(2):
                            src = _ap(
                                stg1,
                                cc * 16 * W + hp * W + wp,
                                [[STG_F, 128], [32, 4], [2 * W, HB], [2, WB]],
                            )
                            dst = _ap(
                                blk,
                                ((cc * 2 + hp) * 2 + wp) * 512,
                                [[IN1BLK_F, 128], [128, 4], [16, HB], [1, WB]],
                            )
                            nc.gpsimd.tensor_copy(dst, src)
                return blk

            # prologue: band groups 0-3, first in1 slab
            for g in range(4):
                build_group(g)
            in1blk = build_in1_slab(0)
            if debug:
                nc.sync.dma_start(
                    bass.AP(tensor=dbg["bands"], offset=0, ap=[[BANDS_F, 128], [1, BANDS_F]]),
                    bands[:, :],
                )
                nc.sync.dma_start(
                    bass.AP(tensor=dbg["in1blk"], offset=0, ap=[[IN1BLK_F, 128], [1, IN1BLK_F]]),
                    in1blk[:, :],
                )

            for k, a in enumerate(range(0, HE, HB)):  # 6 block-rows
                out_t = [
                    outp.tile([128, 16 * W], DT.float32, name=f"outt{dc}")
                    for dc in range(4)
                ]
                pieces = _row_pieces(a)
                for hp in range(2):
                    for wp in range(2):
                        for wb in range(4):
                            ps_pieces = [
                                psp.tile([128, 504], DT.float32, name="ps_a"),
                                psp.tile([128, 504], DT.float32, name="ps_b"),
                            ]
                            # one PSUM accumulation group per bank: start
                            # only on the bank's first write, stop on its last
                            bank_pieces = {0: [], 1: []}
                            for i0, ni in pieces:
                                bank_pieces[0 if i0 < 14 else 1].append((i0, ni))
                            for cc in range(CC):
                                lhsT = _ap(
                                    in1blk,
                                    (((cc * 2 + hp) * 2 + wp) * 4 + wb) * 128,
                                    [[IN1BLK_F, 128], [1, 128]],
                                )
                                for i0, ni in pieces:
                                    s0 = (a + i0) % NSLOT
                                    rhs = _ap(
                                        bands,
                                        _band_base(cc, hp, wp, wb) + s0 * WIN_W,
                                        [[BANDS_F, 128], [1, ni * WIN_W]],
                                    )
                                    pi = 0 if i0 < 14 else 1
                                    c0 = (i0 - 14 * pi) * WIN_W
                                    bp = bank_pieces[pi]
                                    nc.tensor.matmul(
                                        ps_pieces[pi][:, c0 : c0 + ni * WIN_W],
                                        lhsT,
                                        rhs,
                                        start=(cc == 0 and (i0, ni) == bp[0]),
                                        stop=(cc == CC - 1 and (i0, ni) == bp[-1]),
                                    )
                            # band -> SBUF -> DRAM
                            band_sb = bandsbp.tile([128, FB], DT.float32, name="band_sb")
                            nc.vector.tensor_copy(band_sb[:, 0:504], ps_pieces[0][:, :])
                            nc.vector.tensor_copy(
                                band_sb[:, 504:1008], ps_pieces[1][:, :]
                            )
                            if debug and (a, hp, wp, wb) == (0, 0, 0, 0):
                                nc.sync.dma_start(
                                    bass.AP(tensor=dbg["band_sb"], offset=0, ap=[[FB, 128], [1, FB]]),
                                    band_sb[:, :],
                                )
                            if debug and (a, hp, wp, wb) == (16, 0, 0, 0):
                                nc.sync.dma_start(
                                    bass.AP(tensor=dbg["band_sb2"], offset=0, ap=[[FB, 128], [1, FB]]),
                                    band_sb[:, :],
                                )
                            bdram = dramp.tile([128, FB], DT.float32, name="bdram")
                            nc.sync.dma_start(bdram[:, :], band_sb[:, :])
                            # diagonal gather DRAM -> ALIGNED[pixel, 441]
                            alig = aligp.tile([128, ND], DT.float32, name="alig")
                            for he in range(HB):
                                src = _ap(
                                    bdram,
                                    he * (16 * FB + WIN_W),
                                    [[FB + 1, 16], [WIN_W, NP], [1, NP]],
                                )
                                dst = _ap(
                                    alig,
                                    he * 16 * ND,
                                    [[ND, 16], [NP, NP], [1, NP]],
                                )
                                eng = nc.sync if he % 2 == 0 else nc.scalar
                                eng.dma_start(dst, src)
                            if debug and (a, hp, wp, wb) == (0, 0, 0, 0):
                                nc.sync.dma_start(
                                    bass.AP(tensor=dbg["alig"], offset=0, ap=[[ND, 128], [1, ND]]),
                                    alig[:, :],
                                )
                            if debug and (a, hp, wp, wb) == (16, 0, 0, 0):
                                nc.sync.dma_start(
                                    bass.AP(tensor=dbg["alig2"], offset=0, ap=[[ND, 128], [1, ND]]),
                                    alig[:, :],
                                )
                            # transpose pixel-major -> d-major
                            tr = trpp.tile([128, 512], DT.float32, name="tr")
                            for dc in range(4):
                                dlo = dc * 128
                                nd = min(128, ND - dlo)
                                nc.tensor.transpose(
                                    tr[0:nd, dc * 128 : dc * 128 + 128],
                                    alig[:, dlo : dlo + nd],
                                    identity[:, :],
                                )
                            if debug and (a, hp, wp, wb) == (0, 0, 0, 0):
                                nc.vector.tensor_copy(band_sb[:, 0:512], tr[:, :])
                                nc.sync.dma_start(
                                    bass.AP(tensor=dbg["tr"], offset=0, ap=[[512, 128], [1, 512]]),
                                    band_sb[:, 0:512],
                                )
                            # epilogue: leaky(x/C) = 0.1*x/C + relu(0.9*x/C)
                            relu_sb = relup.tile([128, 512], DT.float32, name="relu_sb")
                            for dc in range(4):
                                dlo = dc * 128
                                nd = min(128, ND - dlo)
                                nc.scalar.activation(
                                    relu_sb[0:nd, dc * 128 : dc * 128 + 128],
                                    tr[0:nd, dc * 128 : dc * 128 + 128],
                                    mybir.ActivationFunctionType.Relu,
                                    bias=0.0,
                                    scale=0.9 / C,
                                )
                            for dc in range(4):
                                dlo = dc * 128
                                nd = min(128, ND - dlo)
                                t_ap = _ap(
                                    tr, dc * 128, [[512, nd], [16, HB], [1, WB]]
                                )
                                r_ap = _ap(
                                    relu_sb, dc * 128, [[512, nd], [16, HB], [1, WB]]
                                )
                                dst = _ap(
                                    out_t[dc],
                                    hp * W + 32 * wb + wp,
                                    [[16 * W, nd], [2 * W, HB], [2, WB]],
                                )
                                nc.vector.scalar_tensor_tensor(
                                    dst,
                                    t_ap,
                                    0.1 / C,
                                    r_ap,
                                    mybir.AluOpType.mult,
                                    mybir.AluOpType.add,
                                )
                # prefetch next band group / in1 slab
                if 4 + k < 9:
                    build_group(4 + k)
                if k + 1 < 6:
                    in1blk = build_in1_slab(k + 1)
                # stores for this block-row
                for dc in range(4):
                    dlo = dc * 128
                    nd = min(128, ND - dlo)
                    dst = bass.AP(
                        tensor=out_d,
                        offset=dlo * HW + (2 * a) * W,
                        ap=[[HW, nd], [1, 16 * W]],
                    )
                    nc.sync.dma_start(dst, out_t[dc][0:nd, :])
            if debug:
                nc.sync.dma_start(
                    bass.AP(tensor=dbg["bands2"], offset=0, ap=[[BANDS_F, 128], [1, BANDS_F]]),
                    bands[:, :],
                )

    if waitsplit:
        _split_excess_waits(nc)
    return nc


_NC_CACHE = None


def _get_nc():
    global _NC_CACHE
    if _NC_CACHE is None:
        _NC_CACHE = _build_nc()
    return _NC_CACHE


def kernel(input1, input2):
    input1 = np.ascontiguousarray(np.asarray(input1, dtype=np.float32))
    input2 = np.ascontiguousarray(np.asarray(input2, dtype=np.float32))
    assert input1.shape == (B, C, H, W) and input2.shape == (B, C, H, W)
    nc = _get_nc()
    in_maps = [{"in1": input1[b], "in2": input2[b]} for b in range(B)]
    res = run_bass_kernel_spmd(nc, in_maps, core_ids=list(range(B)))
    return np.stack([res.results[b]["out"] for b in range(B)], axis=0)



# revision 19
# speedup vs baseline: 1.4254x; 1.4254x over previous
"""Trainium2 Bass kernel for nn_Correlation (FlowNet-style cost volume).

Problem: input1/input2 [8, 256, 96, 128] f32 ->
         out [8, 441, 96, 128] f32
  out[b, 21*i+j, h, w] = leaky_relu_0.1( (1/256) * sum_c
        in1[b,c,h,w] * in2pad[b,c, h+2i, w+2j] )       (pad 20 each side)

Strategy (data-parallel over B across 8 cores; per core = 1 sample):
  * in2 kept in SBUF as a zero-padded full-res image [136, 168] (bf16,
    per 128-channel chunk). Parity selection happens in the matmul APs
    (stride-2 free dims) -- no on-chip rearrangement at all.
  * Pixel tile = 128 pixels (8 parity rows x 16 parity cols) per
    (block-row k, h-parity hp, w-parity wp, w-block wb). PE computes
    band[pixel, (r, v)] over the 28x36 dilated window (contract C=256,
    bf16, fp32 PSUM, two 504-col PSUM banks).
  * leaky_relu = max(0.1x, x) fused into the PSUM->SBUF pass (DVE stt /
    scalar ACT Lrelu); 1/256 pre-folded into in1 (power of two, exact).
  * The per-pixel alignment base (he*36+we) is baked into the DRAM
    scratch *write* AP (per-partition stagger is legal on the flat DRAM
    side), so the gather back is a plain 2-dim AP with 1.5KB contiguous
    runs -- 1 gather per (k,hp,wp,wb) instead of per-(he,wb).
  * Gathered rows hold (i, v=36) windows; PE transpose selects (i, j<21)
    via a 2-free-dim stationary AP and emits d-major chunks of 126.
  * Output bf16, converted to f32 on host.
"""

import numpy as np

import concourse.bass as bass
import concourse.mybir as mybir
from concourse.tile import TileContext
from concourse.bass_utils import run_bass_kernel_spmd
from concourse.masks import make_identity

DT = mybir.dt
AF = mybir.ActivationFunctionType
ALU = mybir.AluOpType

# ---- problem geometry ----
B, C, H, W = 8, 256, 96, 128
NP = 21                      # displacements per axis
ND = NP * NP                 # 441
CC = 2                       # C chunks of 128
HW = H * W

IMG_H, IMG_W = H + 40, W + 40      # 136 x 168 padded full-res image
IMG_F = IMG_H * IMG_W              # 22848
NBK = 6                            # block rows of 16 full-res rows
NR, NV = 28, 36                    # window rows / cols (parity space)
BAND = NR * NV                     # 1008
RUN = (NP - 1) * NV + NP           # 741 contiguous gather run (t = i*36+j)
SPITCH = 1032                      # scratch row pitch (>= 1029: no row overlap)
SBASE = 7 * NV + 15                # 267 stagger headroom
SROWS = 129                        # scratch rows (>= (SBASE+127*SPITCH+BAND)/SPITCH)
STG1_F = CC * 16 * W               # 4096
# transpose d-chunks (i0, ni): nd = 21*ni
CHUNKS = [(0, 6), (6, 6), (12, 6), (18, 3)]

_MAX_WAITS = 1


def _split_excess_waits(nc):
    """This walrus build accepts only ONE sync-wait per instruction; Tile
    emits multi-waits. Hoist excess waits onto same-engine NOPs inserted
    right before the over-subscribed instruction."""
    nid = 0
    for f in nc.m.functions:
        for blk in f.blocks:
            insts = list(blk.instructions)
            out = []
            changed = False
            for inst in insts:
                si = inst.sync_info
                if si is not None and si.on_wait and len(si.on_wait) > _MAX_WAITS:
                    waits = list(si.on_wait)
                    extra, keep = waits[:-_MAX_WAITS], waits[-_MAX_WAITS:]
                    for k in range(0, len(extra), _MAX_WAITS):
                        nop = mybir.InstNoOp(name=f"I-waitsplit-{nid}", ins=[], outs=[])
                        nid += 1
                        nop.engine = inst.engine
                        nop.sync_info = mybir.SyncInfo(
                            on_wait=extra[k : k + _MAX_WAITS], on_update=[]
                        )
                        out.append(nop)
                        changed = True
                    si.on_wait = keep
                    inst.sync_info = si
                out.append(inst)
            if changed:
                blk.instructions = out
    return nc


def _ap(t, off_extra, dims):
    return bass.AP(tensor=t.tensor, offset=t.offset + off_extra, ap=dims)


def _build_nc(waitsplit=True, use_lrelu=False):
    nc = bass.Bass()
    in1_d = nc.dram_tensor("in1", [C, H, W], DT.float32, kind="ExternalInput")
    in2_d = nc.dram_tensor("in2", [C, H, W], DT.float32, kind="ExternalInput")
    out_d = nc.dram_tensor("out", [ND, H, W], DT.bfloat16, kind="ExternalOutput")

    with TileContext(nc) as tc:
        with (
            tc.tile_pool(name="constp", bufs=1) as constp,
            tc.tile_pool(name="stg1p", bufs=2) as stg1p,
            tc.tile_pool(name="bandp", bufs=3) as bandp,
            tc.tile_pool(name="aligp", bufs=3) as aligp,
            tc.tile_pool(name="outp", bufs=2) as outp,
            tc.tile_pool(name="psp", bufs=2, space="PSUM") as psp,
            tc.tile_pool(name="trpp", bufs=2, space="PSUM") as trpp,
            tc.tile_pool(name="dramp", bufs=10, space="DRAM") as dramp,
        ):
            ident = constp.tile([128, 128], DT.bfloat16)
            make_identity(nc, ident)

            # persistent padded in2 images (one per 128-channel chunk)
            img = [constp.tile([128, IMG_F], DT.bfloat16, name=f"img{cc}") for cc in range(CC)]
            for cc in range(CC):
                t = img[cc]
                # zero borders: top/bottom row bands, left/right col bands
                nc.vector.memset(_ap(t, 0, [[IMG_F, 128], [1, 20 * IMG_W]]), 0.0)
                nc.vector.memset(
                    _ap(t, (IMG_H - 20) * IMG_W, [[IMG_F, 128], [1, 20 * IMG_W]]), 0.0
                )
                nc.vector.memset(
                    _ap(t, 20 * IMG_W, [[IMG_F, 128], [IMG_W, 96], [1, 20]]), 0.0
                )
                nc.vector.memset(
                    _ap(t, 20 * IMG_W + 20 + W, [[IMG_F, 128], [IMG_W, 96], [1, 20]]),
                    0.0,
                )

            def load_img_group(g):
                """in2 full-res rows [16g, 16g+16) -> img rows [16g+20, ...)."""
                for cc in range(CC):
                    nc.gpsimd.dma_start(
                        _ap(
                            img[cc],
                            (16 * g + 20) * IMG_W + 20,
                            [[IMG_F, 128], [IMG_W, 16], [1, W]],
                        ),
                        in2_d[cc * 128 : (cc + 1) * 128, 16 * g : 16 * g + 16, :],
                    )

            def load_stg1(k):
                """in1 rows [16k, 16k+16), blocked to [c, (cc,hp,wp,wb)*128
                pixels] weight tiles, prescaled by 1/C (fold into the copy)."""
                t = stg1p.tile([128, STG1_F], DT.bfloat16, name="stg1")
                for cc in range(CC):
                    nc.gpsimd.dma_start(
                        _ap(t, cc * 16 * W, [[STG1_F, 128], [1, 16 * W]]),
                        in1_d[cc * 128 : (cc + 1) * 128, 16 * k : 16 * k + 16, :],
                    )
                blk = stg1p.tile([128, STG1_F], DT.bfloat16, name="in1blk")
                n = 0
                for cc in range(CC):
                    for hp in range(2):
                        for wp in range(2):
                            for wb in range(4):
                                src = _ap(
                                    t,
                                    cc * 16 * W + hp * W + 32 * wb + wp,
                                    [[STG1_F, 128], [2 * W, 8], [2, 16]],
                                )
                                dst = _ap(
                                    blk,
                                    (((cc * 2 + hp) * 2 + wp) * 4 + wb) * 128,
                                    [[STG1_F, 128], [1, 128]],
                                )
                                if n % 2 == 0:
                                    nc.vector.tensor_scalar_mul(dst, src, 1.0 / C)
                                else:
                                    nc.scalar.activation(
                                        dst, src, AF.Copy, scale=1.0 / C
                                    )
                                n += 1
                return blk

            # prologue loads
            for g in range(4):
                load_img_group(g)
            stg1 = {0: load_stg1(0)}

            NS = NBK * 4  # 24 supertiles, s = k*4 + hp*2 + wp
            state = {}  # s -> dict with scratch tiles / alig / meta
            out_t = {}

            def stage_a(s):
                k, hp, wp = s // 4, (s // 2) % 2, s % 2
                st = {"k": k, "hp": hp, "wp": wp, "scr": []}
                for wb in range(4):
                    ps = psp.tile([128, 1024], DT.float32, name="ps")
                    for bank in range(2):
                        for cc in range(CC):
                            lhsT = _ap(
                                stg1[k],
                                (((cc * 2 + hp) * 2 + wp) * 4 + wb) * 128,
                                [[STG1_F, 128], [1, 128]],
                            )
                            rhs = _ap(
                                img[cc],
                                (16 * k + hp + 2 * 14 * bank) * IMG_W + 32 * wb + wp,
                                [[IMG_F, 128], [2 * IMG_W, 14], [2, NV]],
                            )
                            nc.tensor.matmul(
                                _ap(ps, 512 * bank, [[1024, 128], [1, 504]]),
                                lhsT,
                                rhs,
                                start=(cc == 0),
                                stop=(cc == CC - 1),
                            )
                    band = bandp.tile([128, BAND], DT.bfloat16, name="band")
                    src = _ap(ps, 0, [[1024, 128], [512, 2], [1, 504]])
                    dst = _ap(band, 0, [[BAND, 128], [504, 2], [1, 504]])
                    if wb % 2 == 0:
                        nc.vector.tensor_copy(dst, src)
                    else:
                        nc.scalar.activation(dst, src, AF.Copy)
                    scr = dramp.tile([SROWS, SPITCH], DT.bfloat16, name="scr")
                    wdst = _ap(
                        scr,
                        SBASE,
                        [[16 * SPITCH - NV, 8], [SPITCH - 1, 16], [1, BAND]],
                    )
                    nc.sync.dma_start(wdst, band[:, :])
                    st["scr"].append(scr)
                state[s] = st

            def stage_b(s):
                st = state[s]
                alig = aligp.tile([128, 4 * RUN], DT.bfloat16, name="alig")
                for wb in range(4):
                    gsrc = _ap(st["scr"][wb], SBASE, [[SPITCH, 128], [1, RUN]])
                    gdst = _ap(alig, wb * RUN, [[4 * RUN, 128], [1, RUN]])
                    eng = nc.gpsimd if wb % 2 else nc.scalar
                    eng.dma_start(gdst, gsrc)
                st["alig"] = alig

            def stage_c(s):
                st = state.pop(s)
                k, hp, wp = st["k"], st["hp"], st["wp"]
                alig = st["alig"]
                if s % 4 == 0:
                    out_t[k] = outp.tile([128, 4 * 16 * W], DT.bfloat16, name="outt")
                ot = out_t[k]
                for wb in range(4):
                    # compact (i, v<36) -> dense (i, j<21) so the transpose
                    # stationary APs are single-free-dim (walrus requirement)
                    dn = aligp.tile([128, ND], DT.bfloat16, name="dense", bufs=6)
                    csrc = _ap(alig, wb * RUN, [[4 * RUN, 128], [NV, NP], [1, NP]])
                    cdst = _ap(dn, 0, [[ND, 128], [1, ND]])
                    # leaky_relu fused here: SBUF dual-read is legal (PSUM isn't)
                    if wb % 2 == 0 or not use_lrelu:
                        nc.vector.scalar_tensor_tensor(
                            cdst, csrc, 0.1, csrc, ALU.mult, ALU.max
                        )
                    else:
                        nc.scalar.activation(cdst, csrc, AF.Lrelu, alpha=0.1)
                    tr = trpp.tile([128, 512], DT.bfloat16, name="tr")
                    for c, (i0, ni) in enumerate(CHUNKS):
                        nd = ni * NP
                        tin = _ap(dn, i0 * NP, [[ND, 128], [1, nd]])
                        nc.tensor.transpose(
                            _ap(tr, c * 128, [[512, nd], [1, 128]]),
                            tin,
                            ident[:, :],
                        )
                    for c, (i0, ni) in enumerate(CHUNKS):
                        nd = ni * NP
                        src = _ap(tr, c * 128, [[512, nd], [1, 128]])
                        dst = _ap(
                            ot,
                            c * 16 * W + hp * W + 32 * wb + wp,
                            [[4 * 16 * W, nd], [2 * W, 8], [2, 16]],
                        )
                        if wb % 2 == 0:
                            nc.vector.tensor_copy(dst, src)
                        else:
                            nc.scalar.activation(dst, src, AF.Copy)
                if s % 4 == 3:
                    # store block-row k: chunks 0-2 merged, chunk 3 separate
                    nc.sync.dma_start(
                        bass.AP(
                            tensor=out_d,
                            offset=(16 * k) * W,
                            ap=[[HW, 126], [126 * HW, 3], [1, 16 * W]],
                        ),
                        _ap(ot, 0, [[4 * 16 * W, 126], [16 * W, 3], [1, 16 * W]]),
                    )
                    nc.sync.dma_start(
                        bass.AP(
                            tensor=out_d,
                            offset=378 * HW + (16 * k) * W,
                            ap=[[HW, 63], [1, 16 * W]],
                        ),
                        _ap(ot, 3 * 16 * W, [[4 * 16 * W, 63], [1, 16 * W]]),
                    )

            LAG = 2
            for s in range(NS + LAG):
                if s < NS:
                    k, sub = s // 4, s % 4
                    if sub == 0:
                        if k + 3 < NBK:
                            load_img_group(k + 3)
                        if k + 1 < NBK:
                            stg1[k + 1] = load_stg1(k + 1)
                    stage_a(s)
                if s - 1 >= 0 and s - 1 < NS:
                    stage_b(s - 1)
                if s - LAG >= 0:
                    stage_c(s - LAG)
                    if (s - LAG) % 4 == 3:
                        stg1.pop((s - LAG) // 4, None)

    if waitsplit:
        _split_excess_waits(nc)
    return nc


_NC_CACHE = None


def _get_nc():
    global _NC_CACHE
    if _NC_CACHE is None:
        _NC_CACHE = _build_nc()
    return _NC_CACHE


def kernel(input1, input2):
    input1 = np.ascontiguousarray(np.asarray(input1, dtype=np.float32))
    input2 = np.ascontiguousarray(np.asarray(input2, dtype=np.float32))
    assert input1.shape == (B, C, H, W) and input2.shape == (B, C, H, W)
    nc = _get_nc()
    in_maps = [{"in1": input1[b], "in2": input2[b]} for b in range(B)]
    res = run_bass_kernel_spmd(nc, in_maps, core_ids=list(range(B)))
    return np.stack(
        [np.asarray(res.results[b]["out"]).astype(np.float32) for b in range(B)],
        axis=0,
    )


# revision 23
# speedup vs baseline: 1.5448x; 1.0837x over previous
"""Trainium2 Bass kernel for nn_Correlation (FlowNet-style cost volume).

Problem: input1/input2 [8, 256, 96, 128] f32 ->
         out [8, 441, 96, 128] f32
  out[b, 21*i+j, h, w] = leaky_relu_0.1( (1/256) * sum_c
        in1[b,c,h,w] * in2pad[b,c, h+2i, w+2j] )       (pad 20 each side)

Strategy (data-parallel over B across 8 cores; per core = 1 sample):
  * in2 kept in SBUF as a zero-padded full-res image [136, 168] (bf16,
    per 128-channel chunk). Parity selection happens in the matmul APs
    (stride-2 free dims) -- no on-chip rearrangement at all.
  * Pixel tile = 128 pixels (8 parity rows x 16 parity cols) per
    (block-row k, h-parity hp, w-parity wp, w-block wb). PE computes
    band[pixel, (r, v)] over the 28x36 dilated window (contract C=256,
    bf16, fp32 PSUM, two 504-col PSUM banks).
  * leaky_relu = max(0.1x, x) fused into the PSUM->SBUF pass (DVE stt /
    scalar ACT Lrelu); 1/256 pre-folded into in1 (power of two, exact).
  * The per-pixel alignment base (he*36+we) is baked into the DRAM
    scratch *write* AP (per-partition stagger is legal on the flat DRAM
    side), so the gather back is a plain 2-dim AP with 1.5KB contiguous
    runs -- 1 gather per (k,hp,wp,wb) instead of per-(he,wb).
  * Gathered rows hold (i, v=36) windows; PE transpose selects (i, j<21)
    via a 2-free-dim stationary AP and emits d-major chunks of 126.
  * Output bf16, converted to f32 on host.
"""

import numpy as np

import concourse.bass as bass
import concourse.mybir as mybir
from concourse.tile import TileContext
from concourse.bass_utils import run_bass_kernel_spmd
from concourse.masks import make_identity

DT = mybir.dt
AF = mybir.ActivationFunctionType
ALU = mybir.AluOpType

# ---- problem geometry ----
B, C, H, W = 8, 256, 96, 128
NP = 21                      # displacements per axis
ND = NP * NP                 # 441
CC = 2                       # C chunks of 128
HW = H * W

IMG_H, IMG_W = H + 40, W + 40      # 136 x 168 padded full-res image
IMG_F = IMG_H * IMG_W              # 22848
NBK = 6                            # block rows of 16 full-res rows
NR, NV = 28, 36                    # window rows / cols (parity space)
BAND = NR * NV                     # 1008
RUN = (NP - 1) * NV + NP           # 741 contiguous gather run (t = i*36+j)
SPITCH = 1032                      # scratch row pitch (>= 1029: no row overlap)
SBASE = 7 * NV + 15                # 267 stagger headroom
SROWS = 129                        # scratch rows (>= (SBASE+127*SPITCH+BAND)/SPITCH)
STG1_F = CC * 16 * W               # 4096
# transpose d-chunks (i0, ni): nd = 21*ni
CHUNKS = [(0, 6), (6, 6), (12, 6), (18, 3)]

_MAX_WAITS = 1


def _split_excess_waits(nc):
    """This walrus build accepts only ONE sync-wait per instruction; Tile
    emits multi-waits. Hoist excess waits onto same-engine NOPs inserted
    right before the over-subscribed instruction."""
    nid = 0
    for f in nc.m.functions:
        for blk in f.blocks:
            insts = list(blk.instructions)
            out = []
            changed = False
            for inst in insts:
                si = inst.sync_info
                if si is not None and si.on_wait and len(si.on_wait) > _MAX_WAITS:
                    waits = list(si.on_wait)
                    extra, keep = waits[:-_MAX_WAITS], waits[-_MAX_WAITS:]
                    for k in range(0, len(extra), _MAX_WAITS):
                        nop = mybir.InstNoOp(name=f"I-waitsplit-{nid}", ins=[], outs=[])
                        nid += 1
                        nop.engine = inst.engine
                        nop.sync_info = mybir.SyncInfo(
                            on_wait=extra[k : k + _MAX_WAITS], on_update=[]
                        )
                        out.append(nop)
                        changed = True
                    si.on_wait = keep
                    inst.sync_info = si
                out.append(inst)
            if changed:
                blk.instructions = out
    return nc


def _ap(t, off_extra, dims):
    return bass.AP(tensor=t.tensor, offset=t.offset + off_extra, ap=dims)


def _build_nc(waitsplit=True, use_lrelu=False):
    nc = bass.Bass()
    in1_d = nc.dram_tensor("in1", [C, H, W], DT.float32, kind="ExternalInput")
    in2_d = nc.dram_tensor("in2", [C, H, W], DT.float32, kind="ExternalInput")
    out_d = nc.dram_tensor("out", [ND, H, W], DT.bfloat16, kind="ExternalOutput")

    with TileContext(nc) as tc:
        with (
            tc.tile_pool(name="constp", bufs=1) as constp,
            tc.tile_pool(name="stg1p", bufs=2) as stg1p,
            tc.tile_pool(name="bandp", bufs=4) as bandp,
            tc.tile_pool(name="aligp", bufs=3) as aligp,
            tc.tile_pool(name="outp", bufs=2) as outp,
            tc.tile_pool(name="psp", bufs=3, space="PSUM") as psp,
            tc.tile_pool(name="trpp", bufs=2, space="PSUM") as trpp,
            tc.tile_pool(name="dramp", bufs=10, space="DRAM") as dramp,
        ):
            ident = constp.tile([128, 128], DT.bfloat16)
            make_identity(nc, ident)

            # persistent padded in2 images (one per 128-channel chunk)
            img = [constp.tile([128, IMG_F], DT.bfloat16, name=f"img{cc}") for cc in range(CC)]
            for cc in range(CC):
                t = img[cc]
                # zero borders: top/bottom row bands, left/right col bands
                nc.vector.memset(_ap(t, 0, [[IMG_F, 128], [1, 20 * IMG_W]]), 0.0)
                nc.vector.memset(
                    _ap(t, (IMG_H - 20) * IMG_W, [[IMG_F, 128], [1, 20 * IMG_W]]), 0.0
                )
                nc.vector.memset(
                    _ap(t, 20 * IMG_W, [[IMG_F, 128], [IMG_W, 96], [1, 20]]), 0.0
                )
                nc.vector.memset(
                    _ap(t, 20 * IMG_W + 20 + W, [[IMG_F, 128], [IMG_W, 96], [1, 20]]),
                    0.0,
                )

            def load_img_group(g):
                """in2 full-res rows [16g, 16g+16) -> img rows [16g+20, ...)."""
                for cc in range(CC):
                    nc.gpsimd.dma_start(
                        _ap(
                            img[cc],
                            (16 * g + 20) * IMG_W + 20,
                            [[IMG_F, 128], [IMG_W, 16], [1, W]],
                        ),
                        in2_d[cc * 128 : (cc + 1) * 128, 16 * g : 16 * g + 16, :],
                    )

            def load_stg1(k):
                """in1 rows [16k, 16k+16), blocked to [c, (cc,hp,wp,wb)*128
                pixels] weight tiles, prescaled by 1/C (fold into the copy)."""
                t = stg1p.tile([128, STG1_F], DT.bfloat16, name="stg1")
                for cc in range(CC):
                    nc.gpsimd.dma_start(
                        _ap(t, cc * 16 * W, [[STG1_F, 128], [1, 16 * W]]),
                        in1_d[cc * 128 : (cc + 1) * 128, 16 * k : 16 * k + 16, :],
                    )
                blk = stg1p.tile([128, STG1_F], DT.bfloat16, name="in1blk")
                n = 0
                for cc in range(CC):
                    for hp in range(2):
                        for wp in range(2):
                            for wb in range(4):
                                src = _ap(
                                    t,
                                    cc * 16 * W + hp * W + 32 * wb + wp,
                                    [[STG1_F, 128], [2 * W, 8], [2, 16]],
                                )
                                dst = _ap(
                                    blk,
                                    (((cc * 2 + hp) * 2 + wp) * 4 + wb) * 128,
                                    [[STG1_F, 128], [1, 128]],
                                )
                                if n % 2 == 0:
                                    nc.vector.tensor_scalar_mul(dst, src, 1.0 / C)
                                else:
                                    nc.scalar.activation(
                                        dst, src, AF.Copy, scale=1.0 / C
                                    )
                                n += 1
                return blk

            # prologue loads
            for g in range(4):
                load_img_group(g)
            stg1 = {0: load_stg1(0)}

            NS = NBK * 4  # 24 supertiles, s = k*4 + hp*2 + wp
            state = {}  # s -> dict with scratch tiles / alig / dense / meta
            out_t = {}

            def stage_a_wb(s, wb):
                """4 matmuls + band copy + scratch write for one wb tile."""
                k, hp, wp = s // 4, (s // 2) % 2, s % 2
                st = state.setdefault(s, {"k": k, "hp": hp, "wp": wp, "scr": [], "dn": []})
                ps = psp.tile([128, 1024], DT.float32, name="ps")
                for bank in range(2):
                    for cc in range(CC):
                        lhsT = _ap(
                            stg1[k],
                            (((cc * 2 + hp) * 2 + wp) * 4 + wb) * 128,
                            [[STG1_F, 128], [1, 128]],
                        )
                        rhs = _ap(
                            img[cc],
                            (16 * k + hp + 2 * 14 * bank) * IMG_W + 32 * wb + wp,
                            [[IMG_F, 128], [2 * IMG_W, 14], [2, NV]],
                        )
                        nc.tensor.matmul(
                            _ap(ps, 512 * bank, [[1024, 128], [1, 504]]),
                            lhsT,
                            rhs,
                            start=(cc == 0),
                            stop=(cc == CC - 1),
                        )
                band = bandp.tile([128, BAND], DT.bfloat16, name="band")
                src = _ap(ps, 0, [[1024, 128], [512, 2], [1, 504]])
                dst = _ap(band, 0, [[BAND, 128], [504, 2], [1, 504]])
                if wb % 2 == 0:
                    nc.vector.tensor_copy(dst, src)
                else:
                    nc.scalar.activation(dst, src, AF.Copy)
                scr = dramp.tile([SROWS, SPITCH], DT.bfloat16, name="scr")
                wdst = _ap(
                    scr,
                    SBASE,
                    [[16 * SPITCH - NV, 8], [SPITCH - 1, 16], [1, BAND]],
                )
                nc.sync.dma_start(wdst, band[:, :])
                st["scr"].append(scr)

            def stage_gather(s):
                st = state[s]
                alig = aligp.tile([128, 4 * RUN], DT.bfloat16, name="alig")
                for wb in range(4):
                    gsrc = _ap(st["scr"][wb], SBASE, [[SPITCH, 128], [1, RUN]])
                    gdst = _ap(alig, wb * RUN, [[4 * RUN, 128], [1, RUN]])
                    eng = nc.gpsimd if wb % 2 else nc.scalar
                    eng.dma_start(gdst, gsrc)
                st["alig"] = alig

            def stage_compact(s):
                """(i, v<36) -> dense (i, j<21) with fused leaky_relu; the
                transpose stationary APs must be single-free-dim (walrus)."""
                st = state[s]
                alig = st["alig"]
                for wb in range(4):
                    dn = aligp.tile([128, ND], DT.bfloat16, name="dense", bufs=9)
                    csrc = _ap(alig, wb * RUN, [[4 * RUN, 128], [NV, NP], [1, NP]])
                    cdst = _ap(dn, 0, [[ND, 128], [1, ND]])
                    if use_lrelu and wb % 2:
                        nc.scalar.activation(cdst, csrc, AF.Lrelu, alpha=0.1)
                    else:
                        nc.vector.scalar_tensor_tensor(
                            cdst, csrc, 0.1, csrc, ALU.mult, ALU.max
                        )
                    st["dn"].append(dn)

            def stage_tp_wb(s, wb):
                """4 transposes + 4 parity-scatter copies for one wb tile."""
                st = state[s]
                k, hp, wp = st["k"], st["hp"], st["wp"]
                if s % 4 == 0 and wb == 0:
                    out_t[k] = outp.tile([128, 4 * 16 * W], DT.bfloat16, name="outt")
                ot = out_t[k]
                dn = st["dn"][wb]
                tr = trpp.tile([128, 512], DT.bfloat16, name="tr")
                for c, (i0, ni) in enumerate(CHUNKS):
                    nd = ni * NP
                    tin = _ap(dn, i0 * NP, [[ND, 128], [1, nd]])
                    nc.tensor.transpose(
                        _ap(tr, c * 128, [[512, nd], [1, 128]]),
                        tin,
                        ident[:, :],
                    )
                for c, (i0, ni) in enumerate(CHUNKS):
                    nd = ni * NP
                    src = _ap(tr, c * 128, [[512, nd], [1, 128]])
                    dst = _ap(
                        ot,
                        c * 16 * W + hp * W + 32 * wb + wp,
                        [[4 * 16 * W, nd], [2 * W, 8], [2, 16]],
                    )
                    if wb % 2 == 0:
                        nc.vector.tensor_copy(dst, src)
                    else:
                        nc.scalar.activation(dst, src, AF.Copy)

            def stage_store(s):
                k = state[s]["k"]
                ot = out_t[k]
                nc.sync.dma_start(
                    bass.AP(
                        tensor=out_d,
                        offset=(16 * k) * W,
                        ap=[[HW, 126], [126 * HW, 3], [1, 16 * W]],
                    ),
                    _ap(ot, 0, [[4 * 16 * W, 126], [16 * W, 3], [1, 16 * W]]),
                )
                nc.sync.dma_start(
                    bass.AP(
                        tensor=out_d,
                        offset=378 * HW + (16 * k) * W,
                        ap=[[HW, 63], [1, 16 * W]],
                    ),
                    _ap(ot, 3 * 16 * W, [[4 * 16 * W, 63], [1, 16 * W]]),
                )

            LAG = 3
            for s in range(NS + LAG):
                if s < NS:
                    k, sub = s // 4, s % 4
                    if sub == 0:
                        if k + 3 < NBK:
                            load_img_group(k + 3)
                        if k + 1 < NBK:
                            stg1[k + 1] = load_stg1(k + 1)
                # interleave matmuls(s) with transposes(s-LAG) per wb so
                # transpose work fills PSUM-dependency gaps in the PE stream
                for wb in range(4):
                    if s < NS:
                        stage_a_wb(s, wb)
                    if s - LAG >= 0:
                        stage_tp_wb(s - LAG, wb)
                if 0 <= s - 1 < NS:
                    stage_gather(s - 1)
                if 0 <= s - 2 < NS:
                    stage_compact(s - 2)
                if s - LAG >= 0:
                    if (s - LAG) % 4 == 3:
                        stage_store(s - LAG)
                        stg1.pop((s - LAG) // 4, None)
                    state.pop(s - LAG)

    if waitsplit:
        _split_excess_waits(nc)
    return nc


_NC_CACHE = None


def _get_nc():
    global _NC_CACHE
    if _NC_CACHE is None:
        _NC_CACHE = _build_nc()
    return _NC_CACHE


def kernel(input1, input2):
    input1 = np.ascontiguousarray(np.asarray(input1, dtype=np.float32))
    input2 = np.ascontiguousarray(np.asarray(input2, dtype=np.float32))
    assert input1.shape == (B, C, H, W) and input2.shape == (B, C, H, W)
    nc = _get_nc()
    in_maps = [{"in1": input1[b], "in2": input2[b]} for b in range(B)]
    res = run_bass_kernel_spmd(nc, in_maps, core_ids=list(range(B)))
    return np.stack(
        [np.asarray(res.results[b]["out"]).astype(np.float32) for b in range(B)],
        axis=0,
    )


# revision 27
# speedup vs baseline: 1.5520x; 1.0047x over previous
"""Trainium2 Bass kernel for nn_Correlation (FlowNet-style cost volume).

Problem: input1/input2 [8, 256, 96, 128] f32 ->
         out [8, 441, 96, 128] f32
  out[b, 21*i+j, h, w] = leaky_relu_0.1( (1/256) * sum_c
        in1[b,c,h,w] * in2pad[b,c, h+2i, w+2j] )       (pad 20 each side)

Strategy (data-parallel over B across 8 cores; per core = 1 sample):
  * in2 kept in SBUF as a zero-padded full-res image [136, 168] (bf16,
    per 128-channel chunk). Parity selection happens in the matmul APs
    (stride-2 free dims) -- no on-chip rearrangement at all.
  * Pixel tile = 128 pixels (8 parity rows x 16 parity cols) per
    (block-row k, h-parity hp, w-parity wp, w-block wb). PE computes
    band[pixel, (r, v)] over the 28x36 dilated window (contract C=256,
    bf16, fp32 PSUM, two 504-col PSUM banks).
  * leaky_relu = max(0.1x, x) fused into the PSUM->SBUF pass (DVE stt /
    scalar ACT Lrelu); 1/256 pre-folded into in1 (power of two, exact).
  * The per-pixel alignment base (he*36+we) is baked into the DRAM
    scratch *write* AP (per-partition stagger is legal on the flat DRAM
    side), so the gather back is a plain 2-dim AP with 1.5KB contiguous
    runs -- 1 gather per (k,hp,wp,wb) instead of per-(he,wb).
  * Gathered rows hold (i, v=36) windows; PE transpose selects (i, j<21)
    via a 2-free-dim stationary AP and emits d-major chunks of 126.
  * Output bf16, converted to f32 on host.
"""

import numpy as np

import concourse.bass as bass
import concourse.mybir as mybir
from concourse.tile import TileContext
from concourse.bass_utils import run_bass_kernel_spmd
from concourse.masks import make_identity

DT = mybir.dt
AF = mybir.ActivationFunctionType
ALU = mybir.AluOpType

# ---- problem geometry ----
B, C, H, W = 8, 256, 96, 128
NP = 21                      # displacements per axis
ND = NP * NP                 # 441
CC = 2                       # C chunks of 128
HW = H * W

IMG_H, IMG_W = H + 40, W + 40      # 136 x 168 padded full-res image
IMG_F = IMG_H * IMG_W              # 22848
NBK = 6                            # block rows of 16 full-res rows
NR, NV = 28, 36                    # window rows / cols (parity space)
BAND = NR * NV                     # 1008
RUN = (NP - 1) * NV + NP           # 741 contiguous gather run (t = i*36+j)
SPITCH = 1032                      # scratch row pitch (>= 1029: no row overlap)
SBASE = 7 * NV + 15                # 267 stagger headroom
SROWS = 129                        # scratch rows (>= (SBASE+127*SPITCH+BAND)/SPITCH)
STG1_F = CC * 16 * W               # 4096
# transpose d-chunks (i0, ni): nd = 21*ni
CHUNKS = [(0, 6), (6, 6), (12, 6), (18, 3)]

_MAX_WAITS = 1


def _split_excess_waits(nc):
    """This walrus build accepts only ONE sync-wait per instruction; Tile
    emits multi-waits. Hoist excess waits onto same-engine NOPs inserted
    right before the over-subscribed instruction."""
    nid = 0
    for f in nc.m.functions:
        for blk in f.blocks:
            insts = list(blk.instructions)
            out = []
            changed = False
            for inst in insts:
                si = inst.sync_info
                if si is not None and si.on_wait and len(si.on_wait) > _MAX_WAITS:
                    waits = list(si.on_wait)
                    extra, keep = waits[:-_MAX_WAITS], waits[-_MAX_WAITS:]
                    for k in range(0, len(extra), _MAX_WAITS):
                        nop = mybir.InstNoOp(name=f"I-waitsplit-{nid}", ins=[], outs=[])
                        nid += 1
                        nop.engine = inst.engine
                        nop.sync_info = mybir.SyncInfo(
                            on_wait=extra[k : k + _MAX_WAITS], on_update=[]
                        )
                        out.append(nop)
                        changed = True
                    si.on_wait = keep
                    inst.sync_info = si
                out.append(inst)
            if changed:
                blk.instructions = out
    return nc


def _ap(t, off_extra, dims):
    return bass.AP(tensor=t.tensor, offset=t.offset + off_extra, ap=dims)


def _build_nc(waitsplit=True, use_lrelu=False):
    nc = bass.Bass()
    in1_d = nc.dram_tensor("in1", [C, H, W], DT.float32, kind="ExternalInput")
    in2_d = nc.dram_tensor("in2", [C, H, W], DT.float32, kind="ExternalInput")
    out_d = nc.dram_tensor("out", [ND, H, W], DT.bfloat16, kind="ExternalOutput")

    with TileContext(nc) as tc:
        with (
            tc.tile_pool(name="constp", bufs=1) as constp,
            tc.tile_pool(name="stg1p", bufs=2) as stg1p,
            tc.tile_pool(name="bandp", bufs=4) as bandp,
            tc.tile_pool(name="aligp", bufs=3) as aligp,
            tc.tile_pool(name="outp", bufs=2) as outp,
            tc.tile_pool(name="psp", bufs=3, space="PSUM") as psp,
            tc.tile_pool(name="trpp", bufs=2, space="PSUM") as trpp,
            tc.tile_pool(name="dramp", bufs=10, space="DRAM") as dramp,
        ):
            ident = constp.tile([128, 128], DT.bfloat16)
            make_identity(nc, ident)

            # persistent padded in2 images (one per 128-channel chunk)
            img = [constp.tile([128, IMG_F], DT.bfloat16, name=f"img{cc}") for cc in range(CC)]
            for cc in range(CC):
                t = img[cc]
                # zero borders: top/bottom row bands, left/right col bands
                nc.vector.memset(_ap(t, 0, [[IMG_F, 128], [1, 20 * IMG_W]]), 0.0)
                nc.vector.memset(
                    _ap(t, (IMG_H - 20) * IMG_W, [[IMG_F, 128], [1, 20 * IMG_W]]), 0.0
                )
                nc.vector.memset(
                    _ap(t, 20 * IMG_W, [[IMG_F, 128], [IMG_W, 96], [1, 20]]), 0.0
                )
                nc.vector.memset(
                    _ap(t, 20 * IMG_W + 20 + W, [[IMG_F, 128], [IMG_W, 96], [1, 20]]),
                    0.0,
                )

            def load_img_group(g):
                """in2 full-res rows [16g, 16g+16) -> img rows [16g+20, ...)."""
                for cc in range(CC):
                    nc.gpsimd.dma_start(
                        _ap(
                            img[cc],
                            (16 * g + 20) * IMG_W + 20,
                            [[IMG_F, 128], [IMG_W, 16], [1, W]],
                        ),
                        in2_d[cc * 128 : (cc + 1) * 128, 16 * g : 16 * g + 16, :],
                    )

            def load_stg1(k):
                """in1 rows [16k, 16k+16) raw row-major staging."""
                t = stg1p.tile([128, STG1_F], DT.bfloat16, name="stg1")
                for cc in range(CC):
                    nc.gpsimd.dma_start(
                        _ap(t, cc * 16 * W, [[STG1_F, 128], [1, 16 * W]]),
                        in1_d[cc * 128 : (cc + 1) * 128, 16 * k : 16 * k + 16, :],
                    )
                return t

            def build_blk(t):
                """Block stg1 into [c, (cc,hp,wp,wb)*128 pixels] weight tiles,
                prescaled by 1/C (folded into the copy)."""
                blk = stg1p.tile([128, STG1_F], DT.bfloat16, name="in1blk")
                n = 0
                for cc in range(CC):
                    for hp in range(2):
                        for wp in range(2):
                            for wb in range(4):
                                src = _ap(
                                    t,
                                    cc * 16 * W + hp * W + 32 * wb + wp,
                                    [[STG1_F, 128], [2 * W, 8], [2, 16]],
                                )
                                dst = _ap(
                                    blk,
                                    (((cc * 2 + hp) * 2 + wp) * 4 + wb) * 128,
                                    [[STG1_F, 128], [1, 128]],
                                )
                                if n % 2 == 0:
                                    nc.vector.tensor_scalar_mul(dst, src, 1.0 / C)
                                else:
                                    nc.scalar.activation(
                                        dst, src, AF.Copy, scale=1.0 / C
                                    )
                                n += 1
                return blk

            # prologue loads
            for g in range(4):
                load_img_group(g)
            raw1 = {0: load_stg1(0)}
            stg1 = {0: build_blk(raw1[0])}

            NS = NBK * 4  # 24 supertiles, s = k*4 + hp*2 + wp
            state = {}  # s -> dict with scratch tiles / alig / dense / meta
            out_t = {}

            def stage_a_wb(s, wb):
                """4 matmuls + band copy + scratch write for one wb tile."""
                k, hp, wp = s // 4, (s // 2) % 2, s % 2
                st = state.setdefault(s, {"k": k, "hp": hp, "wp": wp, "scr": [], "dn": []})
                ps = psp.tile([128, 1024], DT.float32, name="ps")
                for bank in range(2):
                    for cc in range(CC):
                        lhsT = _ap(
                            stg1[k],
                            (((cc * 2 + hp) * 2 + wp) * 4 + wb) * 128,
                            [[STG1_F, 128], [1, 128]],
                        )
                        rhs = _ap(
                            img[cc],
                            (16 * k + hp + 2 * 14 * bank) * IMG_W + 32 * wb + wp,
                            [[IMG_F, 128], [2 * IMG_W, 14], [2, NV]],
                        )
                        nc.tensor.matmul(
                            _ap(ps, 512 * bank, [[1024, 128], [1, 504]]),
                            lhsT,
                            rhs,
                            start=(cc == 0),
                            stop=(cc == CC - 1),
                        )
                band = bandp.tile([128, BAND], DT.bfloat16, name="band")
                src = _ap(ps, 0, [[1024, 128], [512, 2], [1, 504]])
                dst = _ap(band, 0, [[BAND, 128], [504, 2], [1, 504]])
                if wb % 2 == 0:
                    nc.vector.tensor_copy(dst, src)
                else:
                    nc.scalar.activation(dst, src, AF.Copy)
                scr = dramp.tile([SROWS, SPITCH], DT.bfloat16, name="scr")
                wdst = _ap(
                    scr,
                    SBASE,
                    [[16 * SPITCH - NV, 8], [SPITCH - 1, 16], [1, BAND]],
                )
                nc.sync.dma_start(wdst, band[:, :])
                st["scr"].append(scr)

            def stage_gather(s):
                st = state[s]
                alig = aligp.tile([128, 4 * RUN], DT.bfloat16, name="alig")
                for wb in range(4):
                    gsrc = _ap(st["scr"][wb], SBASE, [[SPITCH, 128], [1, RUN]])
                    gdst = _ap(alig, wb * RUN, [[4 * RUN, 128], [1, RUN]])
                    eng = nc.gpsimd if wb % 2 else nc.scalar
                    eng.dma_start(gdst, gsrc)
                st["alig"] = alig

            def stage_compact(s):
                """(i, v<36) -> dense (i, j<21) with fused leaky_relu; the
                transpose stationary APs must be single-free-dim (walrus)."""
                st = state[s]
                alig = st["alig"]
                for wb in range(4):
                    dn = aligp.tile([128, ND], DT.bfloat16, name="dense", bufs=13)
                    csrc = _ap(alig, wb * RUN, [[4 * RUN, 128], [NV, NP], [1, NP]])
                    cdst = _ap(dn, 0, [[ND, 128], [1, ND]])
                    if use_lrelu and wb % 2:
                        nc.scalar.activation(cdst, csrc, AF.Lrelu, alpha=0.1)
                    else:
                        nc.vector.scalar_tensor_tensor(
                            cdst, csrc, 0.1, csrc, ALU.mult, ALU.max
                        )
                    st["dn"].append(dn)

            def stage_tp_wb(s, wb):
                """4 transposes + 4 parity-scatter copies for one wb tile."""
                st = state[s]
                k, hp, wp = st["k"], st["hp"], st["wp"]
                if s % 4 == 0 and wb == 0:
                    out_t[k] = outp.tile([128, 4 * 16 * W], DT.bfloat16, name="outt")
                ot = out_t[k]
                dn = st["dn"][wb]
                tr = trpp.tile([128, 512], DT.bfloat16, name="tr")
                for c, (i0, ni) in enumerate(CHUNKS):
                    nd = ni * NP
                    tin = _ap(dn, i0 * NP, [[ND, 128], [1, nd]])
                    nc.tensor.transpose(
                        _ap(tr, c * 128, [[512, nd], [1, 128]]),
                        tin,
                        ident[:, :],
                    )
                for c, (i0, ni) in enumerate(CHUNKS):
                    nd = ni * NP
                    src = _ap(tr, c * 128, [[512, nd], [1, 128]])
                    dst = _ap(
                        ot,
                        c * 16 * W + hp * W + 32 * wb + wp,
                        [[4 * 16 * W, nd], [2 * W, 8], [2, 16]],
                    )
                    if wb % 2 == 0:
                        nc.vector.tensor_copy(dst, src)
                    else:
                        nc.scalar.activation(dst, src, AF.Copy)

            def stage_store(s):
                k = state[s]["k"]
                ot = out_t[k]
                nc.sync.dma_start(
                    bass.AP(
                        tensor=out_d,
                        offset=(16 * k) * W,
                        ap=[[HW, 126], [126 * HW, 3], [1, 16 * W]],
                    ),
                    _ap(ot, 0, [[4 * 16 * W, 126], [16 * W, 3], [1, 16 * W]]),
                )
                nc.sync.dma_start(
                    bass.AP(
                        tensor=out_d,
                        offset=378 * HW + (16 * k) * W,
                        ap=[[HW, 63], [1, 16 * W]],
                    ),
                    _ap(ot, 3 * 16 * W, [[4 * 16 * W, 63], [1, 16 * W]]),
                )

            LAG = 4
            for s in range(NS + LAG):
                # compacts first: they feed transposes two iterations later,
                # and must sit early in the DVE queue to stay ahead of the PE
                if 0 <= s - 2 < NS:
                    stage_compact(s - 2)
                if s < NS:
                    k, sub = s // 4, s % 4
                    if sub == 0:
                        if k + 3 < NBK:
                            load_img_group(k + 3)
                        if k + 1 < NBK:
                            raw1[k + 1] = load_stg1(k + 1)
                    if sub == 2 and k + 1 < NBK:
                        stg1[k + 1] = build_blk(raw1.pop(k + 1))
                # interleave matmuls(s) with transposes(s-LAG) per wb so
                # transpose work fills PSUM-dependency gaps in the PE stream
                for wb in range(4):
                    if s < NS:
                        stage_a_wb(s, wb)
                    if s - LAG >= 0:
                        stage_tp_wb(s - LAG, wb)
                if 0 <= s - 1 < NS:
                    stage_gather(s - 1)
                if s - LAG >= 0:
                    if (s - LAG) % 4 == 3:
                        stage_store(s - LAG)
                        stg1.pop((s - LAG) // 4, None)
                    state.pop(s - LAG)

    if waitsplit:
        _split_excess_waits(nc)
    return nc


_NC_CACHE = None


def _get_nc():
    global _NC_CACHE
    if _NC_CACHE is None:
        _NC_CACHE = _build_nc()
    return _NC_CACHE


def kernel(input1, input2):
    input1 = np.ascontiguousarray(np.asarray(input1, dtype=np.float32))
    input2 = np.ascontiguousarray(np.asarray(input2, dtype=np.float32))
    assert input1.shape == (B, C, H, W) and input2.shape == (B, C, H, W)
    nc = _get_nc()
    in_maps = [{"in1": input1[b], "in2": input2[b]} for b in range(B)]
    res = run_bass_kernel_spmd(nc, in_maps, core_ids=list(range(B)))
    return np.stack(
        [np.asarray(res.results[b]["out"]).astype(np.float32) for b in range(B)],
        axis=0,
    )


# revision 29
# speedup vs baseline: 1.7485x; 1.1266x over previous
"""Trainium2 Bass kernel for nn_Correlation (FlowNet-style cost volume).

Problem: input1/input2 [8, 256, 96, 128] f32 ->
         out [8, 441, 96, 128] f32
  out[b, 21*i+j, h, w] = leaky_relu_0.1( (1/256) * sum_c
        in1[b,c,h,w] * in2pad[b,c, h+2i, w+2j] )       (pad 20 each side)

Strategy (data-parallel over B across 8 cores; per core = 1 sample):
  * in2 kept in SBUF as a zero-padded full-res image [136, 168] (bf16,
    per 128-channel chunk). Parity selection happens in the matmul APs
    (stride-2 free dims) -- no on-chip rearrangement at all.
  * Pixel tile = 128 pixels (8 parity rows x 16 parity cols) per
    (block-row k, h-parity hp, w-parity wp, w-block wb). PE computes
    band[pixel, (r, v)] over the 28x36 dilated window (contract C=256,
    bf16, fp32 PSUM, two 504-col PSUM banks).
  * leaky_relu = max(0.1x, x) fused into the PSUM->SBUF pass (DVE stt /
    scalar ACT Lrelu); 1/256 pre-folded into in1 (power of two, exact).
  * The per-pixel alignment base (he*36+we) is baked into the DRAM
    scratch *write* AP (per-partition stagger is legal on the flat DRAM
    side), so the gather back is a plain 2-dim AP with 1.5KB contiguous
    runs -- 1 gather per (k,hp,wp,wb) instead of per-(he,wb).
  * Gathered rows hold (i, v=36) windows; PE transpose selects (i, j<21)
    via a 2-free-dim stationary AP and emits d-major chunks of 126.
  * Output bf16, converted to f32 on host.
"""

import numpy as np

import concourse.bass as bass
import concourse.mybir as mybir
from concourse.tile import TileContext
from concourse.bass_utils import run_bass_kernel_spmd
from concourse.masks import make_identity

DT = mybir.dt
AF = mybir.ActivationFunctionType
ALU = mybir.AluOpType

# ---- problem geometry ----
B, C, H, W = 8, 256, 96, 128
NP = 21                      # displacements per axis
ND = NP * NP                 # 441
CC = 2                       # C chunks of 128
HW = H * W

IMG_H, IMG_W = H + 40, W + 40      # 136 x 168 padded full-res image
IMG_F = IMG_H * IMG_W              # 22848
NBK = 6                            # block rows of 16 full-res rows
NR, NV = 28, 36                    # window rows / cols (parity space)
BAND = NR * NV                     # 1008
RUN = (NP - 1) * NV + NP           # 741 contiguous gather run (t = i*36+j)
SPITCH = 1032                      # scratch row pitch (>= 1029: no row overlap)
SBASE = 7 * NV + 15                # 267 stagger headroom
SROWS = 129                        # scratch rows (>= (SBASE+127*SPITCH+BAND)/SPITCH)
STG1_F = CC * 16 * W               # 4096
# transpose d-chunks (i0, ni): nd = 21*ni
CHUNKS = [(0, 6), (6, 6), (12, 6), (18, 3)]

_MAX_WAITS = 1


def _split_excess_waits(nc):
    """This walrus build accepts only ONE sync-wait per instruction; Tile
    emits multi-waits. Hoist excess waits onto same-engine NOPs inserted
    right before the over-subscribed instruction."""
    nid = 0
    for f in nc.m.functions:
        for blk in f.blocks:
            insts = list(blk.instructions)
            out = []
            changed = False
            for inst in insts:
                si = inst.sync_info
                if si is not None and si.on_wait and len(si.on_wait) > _MAX_WAITS:
                    waits = list(si.on_wait)
                    extra, keep = waits[:-_MAX_WAITS], waits[-_MAX_WAITS:]
                    for k in range(0, len(extra), _MAX_WAITS):
                        nop = mybir.InstNoOp(name=f"I-waitsplit-{nid}", ins=[], outs=[])
                        nid += 1
                        nop.engine = inst.engine
                        nop.sync_info = mybir.SyncInfo(
                            on_wait=extra[k : k + _MAX_WAITS], on_update=[]
                        )
                        out.append(nop)
                        changed = True
                    si.on_wait = keep
                    inst.sync_info = si
                out.append(inst)
            if changed:
                blk.instructions = out
    return nc


def _ap(t, off_extra, dims):
    return bass.AP(tensor=t.tensor, offset=t.offset + off_extra, ap=dims)


def _build_nc(waitsplit=True, use_lrelu=False):
    nc = bass.Bass()
    in1_d = nc.dram_tensor("in1", [C, H, W], DT.float32, kind="ExternalInput")
    in2_d = nc.dram_tensor("in2", [C, H, W], DT.float32, kind="ExternalInput")
    out_d = nc.dram_tensor("out", [ND, H, W], DT.bfloat16, kind="ExternalOutput")

    with TileContext(nc) as tc:
        with (
            tc.tile_pool(name="constp", bufs=1) as constp,
            tc.tile_pool(name="stg1p", bufs=2) as stg1p,
            tc.tile_pool(name="bandp", bufs=4) as bandp,
            tc.tile_pool(name="aligp", bufs=3) as aligp,
            tc.tile_pool(name="outp", bufs=2) as outp,
            tc.tile_pool(name="psp", bufs=3, space="PSUM") as psp,
            tc.tile_pool(name="trpp", bufs=2, space="PSUM") as trpp,
            tc.tile_pool(name="dramp", bufs=10, space="DRAM") as dramp,
        ):
            ident = constp.tile([128, 128], DT.bfloat16)
            make_identity(nc, ident)

            # persistent padded in2 images (one per 128-channel chunk)
            img = [constp.tile([128, IMG_F], DT.bfloat16, name=f"img{cc}") for cc in range(CC)]
            for cc in range(CC):
                t = img[cc]
                # zero borders: top/bottom row bands, left/right col bands
                nc.vector.memset(_ap(t, 0, [[IMG_F, 128], [1, 20 * IMG_W]]), 0.0)
                nc.vector.memset(
                    _ap(t, (IMG_H - 20) * IMG_W, [[IMG_F, 128], [1, 20 * IMG_W]]), 0.0
                )
                nc.vector.memset(
                    _ap(t, 20 * IMG_W, [[IMG_F, 128], [IMG_W, 96], [1, 20]]), 0.0
                )
                nc.vector.memset(
                    _ap(t, 20 * IMG_W + 20 + W, [[IMG_F, 128], [IMG_W, 96], [1, 20]]),
                    0.0,
                )

            def load_img_group(g):
                """in2 full-res rows [16g, 16g+16) -> img rows [16g+20, ...)."""
                for cc in range(CC):
                    nc.gpsimd.dma_start(
                        _ap(
                            img[cc],
                            (16 * g + 20) * IMG_W + 20,
                            [[IMG_F, 128], [IMG_W, 16], [1, W]],
                        ),
                        in2_d[cc * 128 : (cc + 1) * 128, 16 * g : 16 * g + 16, :],
                    )

            def load_stg1(k):
                """in1 rows [16k, 16k+16) raw row-major staging."""
                t = stg1p.tile([128, STG1_F], DT.bfloat16, name="stg1")
                for cc in range(CC):
                    nc.gpsimd.dma_start(
                        _ap(t, cc * 16 * W, [[STG1_F, 128], [1, 16 * W]]),
                        in1_d[cc * 128 : (cc + 1) * 128, 16 * k : 16 * k + 16, :],
                    )
                return t

            def build_blk(t):
                """Block stg1 into [c, (cc,hp,wp,wb)*128 pixels] weight tiles,
                prescaled by 1/C (folded into the copy)."""
                blk = stg1p.tile([128, STG1_F], DT.bfloat16, name="in1blk")
                n = 0
                for cc in range(CC):
                    for hp in range(2):
                        for wp in range(2):
                            for wb in range(4):
                                src = _ap(
                                    t,
                                    cc * 16 * W + hp * W + 32 * wb + wp,
                                    [[STG1_F, 128], [2 * W, 8], [2, 16]],
                                )
                                dst = _ap(
                                    blk,
                                    (((cc * 2 + hp) * 2 + wp) * 4 + wb) * 128,
                                    [[STG1_F, 128], [1, 128]],
                                )
                                if n % 2 == 0:
                                    nc.vector.tensor_scalar_mul(dst, src, 1.0 / C)
                                else:
                                    nc.scalar.activation(
                                        dst, src, AF.Copy, scale=1.0 / C
                                    )
                                n += 1
                return blk

            # prologue loads
            for g in range(4):
                load_img_group(g)
            raw1 = {0: load_stg1(0)}
            stg1 = {0: build_blk(raw1[0])}

            NS = NBK * 4  # 24 supertiles, s = k*4 + hp*2 + wp
            state = {}  # s -> dict with scratch tiles / alig / dense / meta
            out_t = {}

            def stage_a_wb(s, wb):
                """4 matmuls + band copy + scratch write for one wb tile."""
                k, hp, wp = s // 4, (s // 2) % 2, s % 2
                st = state.setdefault(s, {"k": k, "hp": hp, "wp": wp, "scr": [], "dn": []})
                ps = psp.tile([128, 1024], DT.float32, name="ps")
                for bank in range(2):
                    for cc in range(CC):
                        lhsT = _ap(
                            stg1[k],
                            (((cc * 2 + hp) * 2 + wp) * 4 + wb) * 128,
                            [[STG1_F, 128], [1, 128]],
                        )
                        rhs = _ap(
                            img[cc],
                            (16 * k + hp + 2 * 14 * bank) * IMG_W + 32 * wb + wp,
                            [[IMG_F, 128], [2 * IMG_W, 14], [2, NV]],
                        )
                        nc.tensor.matmul(
                            _ap(ps, 512 * bank, [[1024, 128], [1, 504]]),
                            lhsT,
                            rhs,
                            start=(cc == 0),
                            stop=(cc == CC - 1),
                        )
                band = bandp.tile([128, BAND], DT.bfloat16, name="band")
                src = _ap(ps, 0, [[1024, 128], [512, 2], [1, 504]])
                dst = _ap(band, 0, [[BAND, 128], [504, 2], [1, 504]])
                if wb % 2 == 0:
                    nc.vector.tensor_copy(dst, src)
                else:
                    nc.scalar.activation(dst, src, AF.Copy)
                scr = dramp.tile([SROWS, SPITCH], DT.bfloat16, name="scr")
                wdst = _ap(
                    scr,
                    SBASE,
                    [[16 * SPITCH - NV, 8], [SPITCH - 1, 16], [1, BAND]],
                )
                # scratch writes ride the gpsimd ring (bf16->bf16, no cast
                # needed) so the latency-critical gathers own sync/scalar
                nc.gpsimd.dma_start(wdst, band[:, :])
                st["scr"].append(scr)

            def stage_gather(s):
                st = state[s]
                alig = aligp.tile([128, 4 * RUN], DT.bfloat16, name="alig")
                for wb in range(4):
                    gsrc = _ap(st["scr"][wb], SBASE, [[SPITCH, 128], [1, RUN]])
                    gdst = _ap(alig, wb * RUN, [[4 * RUN, 128], [1, RUN]])
                    eng = nc.scalar if wb % 2 else nc.sync
                    eng.dma_start(gdst, gsrc)
                st["alig"] = alig

            def stage_compact(s):
                """(i, v<36) -> dense (i, j<21) with fused leaky_relu; the
                transpose stationary APs must be single-free-dim (walrus)."""
                st = state[s]
                alig = st["alig"]
                for wb in range(4):
                    dn = aligp.tile([128, ND], DT.bfloat16, name="dense", bufs=13)
                    csrc = _ap(alig, wb * RUN, [[4 * RUN, 128], [NV, NP], [1, NP]])
                    cdst = _ap(dn, 0, [[ND, 128], [1, ND]])
                    if use_lrelu and wb % 2:
                        nc.scalar.activation(cdst, csrc, AF.Lrelu, alpha=0.1)
                    else:
                        nc.vector.scalar_tensor_tensor(
                            cdst, csrc, 0.1, csrc, ALU.mult, ALU.max
                        )
                    st["dn"].append(dn)

            def stage_tp_wb(s, wb):
                """4 transposes + 4 parity-scatter copies for one wb tile."""
                st = state[s]
                k, hp, wp = st["k"], st["hp"], st["wp"]
                if s % 4 == 0 and wb == 0:
                    out_t[k] = outp.tile([128, 4 * 16 * W], DT.bfloat16, name="outt")
                ot = out_t[k]
                dn = st["dn"][wb]
                tr = trpp.tile([128, 512], DT.bfloat16, name="tr")
                for c, (i0, ni) in enumerate(CHUNKS):
                    nd = ni * NP
                    tin = _ap(dn, i0 * NP, [[ND, 128], [1, nd]])
                    nc.tensor.transpose(
                        _ap(tr, c * 128, [[512, nd], [1, 128]]),
                        tin,
                        ident[:, :],
                    )
                for c, (i0, ni) in enumerate(CHUNKS):
                    nd = ni * NP
                    src = _ap(tr, c * 128, [[512, nd], [1, 128]])
                    dst = _ap(
                        ot,
                        c * 16 * W + hp * W + 32 * wb + wp,
                        [[4 * 16 * W, nd], [2 * W, 8], [2, 16]],
                    )
                    if wb % 2 == 0:
                        nc.vector.tensor_copy(dst, src)
                    else:
                        nc.scalar.activation(dst, src, AF.Copy)

            def stage_store(s):
                k = state[s]["k"]
                ot = out_t[k]
                nc.sync.dma_start(
                    bass.AP(
                        tensor=out_d,
                        offset=(16 * k) * W,
                        ap=[[HW, 126], [126 * HW, 3], [1, 16 * W]],
                    ),
                    _ap(ot, 0, [[4 * 16 * W, 126], [16 * W, 3], [1, 16 * W]]),
                )
                nc.sync.dma_start(
                    bass.AP(
                        tensor=out_d,
                        offset=378 * HW + (16 * k) * W,
                        ap=[[HW, 63], [1, 16 * W]],
                    ),
                    _ap(ot, 3 * 16 * W, [[4 * 16 * W, 63], [1, 16 * W]]),
                )

            LAG = 4
            for s in range(NS + LAG):
                # compacts first: they feed transposes two iterations later,
                # and must sit early in the DVE queue to stay ahead of the PE
                if 0 <= s - 2 < NS:
                    stage_compact(s - 2)
                if s < NS:
                    k, sub = s // 4, s % 4
                    if sub == 0:
                        if k + 3 < NBK:
                            load_img_group(k + 3)
                        if k + 1 < NBK:
                            raw1[k + 1] = load_stg1(k + 1)
                    if sub == 2 and k + 1 < NBK:
                        stg1[k + 1] = build_blk(raw1.pop(k + 1))
                # interleave matmuls(s) with transposes(s-LAG) per wb so
                # transpose work fills PSUM-dependency gaps in the PE stream
                for wb in range(4):
                    if s < NS:
                        stage_a_wb(s, wb)
                    if s - LAG >= 0:
                        stage_tp_wb(s - LAG, wb)
                if 0 <= s - 1 < NS:
                    stage_gather(s - 1)
                if s - LAG >= 0:
                    if (s - LAG) % 4 == 3:
                        stage_store(s - LAG)
                        stg1.pop((s - LAG) // 4, None)
                    state.pop(s - LAG)

    if waitsplit:
        _split_excess_waits(nc)
    return nc


_NC_CACHE = None


def _get_nc():
    global _NC_CACHE
    if _NC_CACHE is None:
        _NC_CACHE = _build_nc()
    return _NC_CACHE


def kernel(input1, input2):
    input1 = np.ascontiguousarray(np.asarray(input1, dtype=np.float32))
    input2 = np.ascontiguousarray(np.asarray(input2, dtype=np.float32))
    assert input1.shape == (B, C, H, W) and input2.shape == (B, C, H, W)
    nc = _get_nc()
    in_maps = [{"in1": input1[b], "in2": input2[b]} for b in range(B)]
    res = run_bass_kernel_spmd(nc, in_maps, core_ids=list(range(B)))
    return np.stack(
        [np.asarray(res.results[b]["out"]).astype(np.float32) for b in range(B)],
        axis=0,
    )
